# revision 20
# baseline (speedup 1.0000x reference)
"""Trainium2 Bass kernel for AffinityNodeLayer (gnn_message_passing).

Math:
  g = vertex @ W_vert.T                      # [N, H*D] = [4096, 512]
  gram[n,m,h] = <g[n,h,:], g[m,h,:]>         # per-head Gram
  e = sum_g leaky_relu(sum_h W_attn[g,h] * gram[:, :, h])    # [N, N]

Key identity: x_v[n,m] = <S_v * g[n], g[m]> where S_v[f] =
W_attn[v, f // D] — each output head is ONE matmul with contract dim
512 between a per-head-scaled copy of g and g itself:

  e[n,m] = sum_{v=0..7} prelu_{0.2}( (S_v ⊙ g[n]) . g[m] )

x_v (and hence e) is SYMMETRIC, so only ~half the 8x8 grid of
512x512-row/col blocks is computed:

  core i computes blocks (i, (i+b) % 8):
    b = 0        diagonal block, all 8 heads
    b = 1..3     full off-diagonal blocks, all 8 heads (mirrored on host)
    b = 4        "band-4" block, HALF the heads (cores 0-3 take heads
                 0-3, cores 4-7 take heads 4-7); the host adds the two
                 half-contributions (core i and core (i+4)%8) together.

Every core runs the IDENTICAL program (SPMD): each core projects ONLY
its own 512-node column chunk of gT = W_vert @ vertex.T (48 matmuls),
then an on-chip AllGather distributes the 8 chunks; the 4 other chunks
each core needs are pulled from the gathered buffer with an indirect
DMA driven by a host-provided per-core chunk-selection table. Which
heads the half block uses is also data (per-core head permutation in
the scale table S), not control flow.

Matmuls run in float32r (fp32 bits at 1 cycle/row for N=512; ~1.5e-4
matmul rel err vs exact fp32 — measured on HW).
"""

import numpy as np

import concourse.bacc as bacc
import concourse.bass as bass
import concourse.mybir as mybir
import concourse.tile as tile
from concourse.bass import ts
from concourse.bass_utils import run_bass_kernel_spmd

# Problem shapes (hardcoded per harness contract)
N_NODES = 4096
IN_FEAT = 1433
N_HEADS = 8
N_HIDDEN = 64
HD = N_HEADS * N_HIDDEN          # 512 features of g
NEG_SLOPE = 0.2

NCORES = 8
CH = 512                         # column-chunk width == rows per core
NCH = N_NODES // CH              # 8 global column chunks
NLOC = 5                         # local column chunks per core (symmetry)
NHALF = N_HEADS // 2             # heads in the half block
NV = N_HEADS                     # scale variants (head-permuted per core)
FPAD = 1536                      # IN_FEAT padded to 12 * 128
KF = FPAD // 128                 # 12 contraction chunks for the projection
KC = HD // 128                   # 4 contraction chunks for the gram matmuls
MM = CH // 128                   # 4 output row chunks per core
GW = KC * CH                     # 2048: flat row width of one gT chunk

F32 = mybir.dt.float32
F32R = mybir.dt.float32r
I32 = mybir.dt.int32

_CACHE = {}


def _build():
    nc = bacc.Bacc("TRN2", target_bir_lowering=False, debug=False,
                   num_devices=NCORES)
    vT = nc.dram_tensor("vT", [FPAD, CH], F32R, kind="ExternalInput")
    wT = nc.dram_tensor("wT", [FPAD, HD], F32R, kind="ExternalInput")
    S = nc.dram_tensor("S", [128, NV * KC], F32, kind="ExternalInput")
    gsel = nc.dram_tensor("gsel", [128, NLOC - 1], I32, kind="ExternalInput")
    out = nc.dram_tensor("out", [CH, NLOC * CH], F32, kind="ExternalOutput")
    # AllGather bounce buffers (internal DRAM; output in the Shared space)
    ccin = nc.dram_tensor("ccin", [128, GW], F32R, kind="Internal")
    ccout = nc.dram_tensor("ccout", [NCH * 128, GW], F32R, kind="Internal",
                           addr_space="Shared")

    with tile.TileContext(nc) as tc:
        with (
            tc.tile_pool(name="const", bufs=1) as const,
            tc.tile_pool(name="apool", bufs=1) as apool,
            tc.tile_pool(name="gtpool", bufs=2) as gtpool,
            tc.tile_pool(name="vpool", bufs=1) as vpool,
            tc.tile_pool(name="epool", bufs=8) as epool,
            tc.tile_pool(name="tpool", bufs=4) as tpool,
            tc.tile_pool(name="psum", bufs=8, space="PSUM") as psum,
        ):
            wsb = const.tile([128, KF, HD], F32R, tag="wsb")
            ssb = const.tile([128, NV * KC], F32, tag="ssb")
            gsl = const.tile([128, NLOC - 1], I32, tag="gsl")
            nc.sync.dma_start(ssb[:], S[:])
            nc.sync.dma_start(gsl[:], gsel[:])

            # ---- own gT chunk: 48 matmuls off this core's vT columns ----
            gt0 = gtpool.tile([128, GW], F32R, tag="gt", name="gt0")
            vts = []
            for k in range(KF):
                vt = vpool.tile([128, CH], F32R, tag=f"vt{k}", name=f"vt{k}")
                # interleave weight loads with vt loads so the first
                # matmul isn't blocked behind the whole 3 MB of wT
                nc.sync.dma_start(wsb[:, k, :], wT[k * 128:(k + 1) * 128, :])
                nc.sync.dma_start(vt[:], vT[k * 128:(k + 1) * 128, :])
                vts.append(vt)
            for m in range(KC):
                ps = psum.tile([128, CH], F32, tag="ps", name=f"pg{m}")
                for k in range(KF):
                    nc.tensor.matmul(
                        ps[:],
                        wsb[:, k, ts(m, 128)],
                        vts[k][:],
                        start=(k == 0), stop=(k == KF - 1))
                nc.vector.tensor_copy(gt0[:, m * CH:(m + 1) * CH], ps[:])

            # ---- AllGather the 8 chunks (overlaps A-build + block 0) ----
            nc.sync.dma_start(ccin[:], gt0[:])
            nc.gpsimd.collective_compute(
                "AllGather",
                mybir.AluOpType.bypass,
                replica_groups=[list(range(NCORES))],
                ins=[ccin[:]],
                outs=[ccout[:]],
            )

            # ---- 8 scaled lhsT variants from the own chunk ----
            A = []
            for v in range(NV):
                a = apool.tile([128, KC, CH], F32R, tag=f"A{v}", name=f"A{v}")
                for k in range(KC):
                    nc.vector.tensor_scalar_mul(
                        a[:, k, :], gt0[:, k * CH:(k + 1) * CH],
                        ssb[:, v * KC + k:v * KC + k + 1])
                A.append(a)

            for j in range(NLOC):
                if j == 0:
                    gt = gt0
                else:
                    # pull global chunk (core + j) % 8 out of the gathered
                    # buffer; the row offsets live in the host-provided
                    # per-core selection table.
                    gt = gtpool.tile([128, GW], F32R, tag="gt",
                                     name=f"gt{j}")
                    nc.gpsimd.indirect_dma_start(
                        out=gt[:],
                        out_offset=None,
                        in_=ccout[:],
                        in_offset=bass.IndirectOffsetOnAxis(
                            ap=gsl[:, j - 1:j], axis=0),
                    )
                # full blocks use all 8 head variants; the half block
                # (j == 4) uses variants 0..3 (per-core head permutation
                # in S makes those this core's half-block heads).
                vset = list(range(N_HEADS)) if j < NLOC - 1 else \
                    list(range(NHALF))
                es = [epool.tile([128, CH], F32, tag="e", name=f"e{j}_{m}")
                      for m in range(MM)]
                for vi, v in enumerate(vset):
                    for m in range(MM):
                        ps = psum.tile([128, CH], F32, tag="ps",
                                       name=f"px{j}_{v}_{m}")
                        for k in range(KC):
                            nc.tensor.matmul(
                                ps[:],
                                A[v][:, k, ts(m, 128)],
                                gt[:, k * CH:(k + 1) * CH],
                                start=(k == 0), stop=(k == KC - 1))
                        if vi == 0:
                            nc.scalar.activation(
                                es[m][:], ps[:],
                                mybir.ActivationFunctionType.Prelu,
                                alpha=NEG_SLOPE)
                        else:
                            t = tpool.tile([128, CH], F32, tag="tmp",
                                           name=f"t{j}_{v}_{m}")
                            nc.scalar.activation(
                                t[:], ps[:],
                                mybir.ActivationFunctionType.Prelu,
                                alpha=NEG_SLOPE)
                            nc.vector.tensor_tensor(
                                es[m][:], es[m][:], t[:], mybir.AluOpType.add)
                for m in range(MM):
                    nc.sync.dma_start(
                        out[m * 128:(m + 1) * 128, j * CH:(j + 1) * CH],
                        es[m][:])
    nc.compile()
    return nc


def _prepare_in_maps(vertex, W_vert, W_attn):
    vertex = np.ascontiguousarray(vertex, dtype=np.float32)
    W_vert = np.ascontiguousarray(W_vert, dtype=np.float32)
    W_attn = np.ascontiguousarray(W_attn, dtype=np.float32)

    vT = np.zeros((FPAD, N_NODES), dtype=np.float32)
    vT[:IN_FEAT] = vertex.T
    wT = np.zeros((FPAD, HD), dtype=np.float32)
    wT[:IN_FEAT] = W_vert.T

    p = np.arange(128)
    head_of = (np.arange(KC * 128).reshape(KC, 128)) // N_HIDDEN

    in_maps = []
    for i in range(NCORES):
        vT_core = np.ascontiguousarray(vT[:, i * CH:(i + 1) * CH])
        # S[p, v*KC + k] = W_attn[head_v, (128k + p) // N_HIDDEN]
        # Per-core head permutation: variants 0..3 must be this core's
        # half-block heads (cores 0-3 -> heads 0-3, cores 4-7 -> 4-7).
        if i < NCORES // 2:
            heads = list(range(N_HEADS))
        else:
            heads = list(range(NHALF, N_HEADS)) + list(range(NHALF))
        S = np.empty((128, NV * KC), dtype=np.float32)
        for v, h in enumerate(heads):
            for k in range(KC):
                S[:, v * KC + k] = W_attn[h, head_of[k, p]]
        # indirect-DMA row offsets into ccout for local cols 1..4:
        # row p of global chunk (i + j) % 8 lives at ((i+j)%8)*128 + p.
        gsel = np.empty((128, NLOC - 1), dtype=np.int32)
        for j in range(1, NLOC):
            gsel[:, j - 1] = ((i + j) % NCH) * 128 + p
        in_maps.append({"vT": vT_core, "wT": wT, "S": S, "gsel": gsel})
    return in_maps


def _gather(results):
    e = np.empty((N_NODES, N_NODES), dtype=np.float32)

    def blk(i, j):
        return results[i]["out"][:, j * CH:(j + 1) * CH]

    for i in range(NCORES):
        # diagonal
        e[i * CH:(i + 1) * CH, i * CH:(i + 1) * CH] = blk(i, 0)
        # full off-diagonal blocks + mirrors
        for b in range(1, NLOC - 1):
            gj = (i + b) % NCH
            e[i * CH:(i + 1) * CH, gj * CH:(gj + 1) * CH] = blk(i, b)
            e[gj * CH:(gj + 1) * CH, i * CH:(i + 1) * CH] = blk(i, b).T
    # band-4 half blocks: e(i, i+4) = half_i + half_{i+4}.T
    for i in range(NCORES // 2):
        ii = i + NCORES // 2
        full = blk(i, NLOC - 1) + blk(ii, NLOC - 1).T
        e[i * CH:(i + 1) * CH, ii * CH:(ii + 1) * CH] = full
        e[ii * CH:(ii + 1) * CH, i * CH:(i + 1) * CH] = full.T
    return e


def run(vertex, W_vert, W_attn, **run_kwargs):
    """Run the kernel; returns (e, BassKernelResults)."""
    if "nc" not in _CACHE:
        _CACHE["nc"] = _build()
    nc = _CACHE["nc"]
    in_maps = _prepare_in_maps(vertex, W_vert, W_attn)
    r = run_bass_kernel_spmd(nc, in_maps, core_ids=list(range(NCORES)),
                             **run_kwargs)
    return _gather(r.results), r


def kernel(vertex, W_vert, W_attn):
    e, _ = run(vertex, W_vert, W_attn)
    return e


# revision 21
# speedup vs baseline: 1.2054x; 1.2054x over previous
"""Trainium2 Bass kernel for AffinityNodeLayer (gnn_message_passing).

Math:
  g = vertex @ W_vert.T                      # [N, H*D] = [4096, 512]
  gram[n,m,h] = <g[n,h,:], g[m,h,:]>         # per-head Gram
  e = sum_g leaky_relu(sum_h W_attn[g,h] * gram[:, :, h])    # [N, N]

Key identity: x_v[n,m] = <S_v * g[n], g[m]> where S_v[f] =
W_attn[v, f // D] — each output head is ONE matmul with contract dim
512 between a per-head-scaled copy of g and g itself:

  e[n,m] = sum_{v=0..7} prelu_{0.2}( (S_v ⊙ g[n]) . g[m] )

x_v (and hence e) is SYMMETRIC, so only ~half the 8x8 grid of
512x512-row/col blocks is computed:

  core i computes blocks (i, (i+b) % 8):
    b = 0        diagonal block, all 8 heads
    b = 1..3     full off-diagonal blocks, all 8 heads (mirrored on host)
    b = 4        "band-4" block, HALF the heads (cores 0-3 take heads
                 0-3, cores 4-7 take heads 4-7); the host adds the two
                 half-contributions (core i and core (i+4)%8) together.

Every core runs the IDENTICAL program (SPMD): 5 column chunks, 4 full
blocks + 1 half block = 36 head-block units each. Which heads the half
block uses is data (extra scale columns in S), not control flow.

Matmuls run in float32r (fp32 bits at 1 cycle/row for N=512; ~1.5e-4
matmul rel err vs exact fp32 — measured on HW).
"""

import numpy as np

import concourse.bacc as bacc
import concourse.mybir as mybir
import concourse.tile as tile
from concourse.bass import ts
from concourse.bass_utils import run_bass_kernel_spmd

# Problem shapes (hardcoded per harness contract)
N_NODES = 4096
IN_FEAT = 1433
N_HEADS = 8
N_HIDDEN = 64
HD = N_HEADS * N_HIDDEN          # 512 features of g
NEG_SLOPE = 0.2

NCORES = 8
CH = 512                         # column-chunk width == rows per core
NCH = N_NODES // CH              # 8 global column chunks
NLOC = 5                         # local column chunks per core (symmetry)
NHALF = N_HEADS // 2             # heads in the half block
# Heads are PERMUTED per core in S (sum over heads is order-invariant)
# so that each core's half-block heads are always variants 0..3 —
# keeping the device program identical across cores with only 8 scaled
# A variants.
NV = N_HEADS
FPAD = 1536                      # IN_FEAT padded to 12 * 128
KF = FPAD // 128                 # 12 contraction chunks for the projection
KC = HD // 128                   # 4 contraction chunks for the gram matmuls
MM = CH // 128                   # 4 output row chunks per core

F32 = mybir.dt.float32
# Matmul operand dtype: float32r (fp32 bits, ~1.5e-4 matmul err) or
# bfloat16 (faster moving-operand streaming, ~2.6e-3 end-to-end err).
import os as _os
MMDT = mybir.dt.bfloat16 if _os.environ.get("KERNEL_MM_DTYPE") == "bf16" \
    else mybir.dt.float32r
NPDT_IN = None  # numpy dtype for vT/wT staging, set below
import ml_dtypes as _mld
NPDT_IN = _mld.bfloat16 if _os.environ.get("KERNEL_MM_DTYPE") == "bf16" \
    else __import__("numpy").float32

_CACHE = {}


def _build():
    nc = bacc.Bacc("TRN2", target_bir_lowering=False, debug=False,
                   num_devices=NCORES)
    vT = nc.dram_tensor("vT", [FPAD, NLOC * CH], MMDT, kind="ExternalInput")
    wT = nc.dram_tensor("wT", [FPAD, HD], MMDT, kind="ExternalInput")
    S = nc.dram_tensor("S", [128, NV * KC], F32, kind="ExternalInput")
    out = nc.dram_tensor("out", [CH, NLOC * CH], F32, kind="ExternalOutput")

    with tile.TileContext(nc) as tc:
        with (
            tc.tile_pool(name="const", bufs=1) as const,
            tc.tile_pool(name="apool", bufs=1) as apool,
            tc.tile_pool(name="gtpool", bufs=2) as gtpool,
            tc.tile_pool(name="vpool", bufs=2) as vpool,
            tc.tile_pool(name="epool", bufs=8) as epool,
            tc.tile_pool(name="tpool", bufs=4) as tpool,
            tc.tile_pool(name="psum", bufs=8, space="PSUM") as psum,
        ):
            wsb = const.tile([128, KF, HD], MMDT, tag="wsb")
            ssb = const.tile([128, NV * KC], F32, tag="ssb")
            nc.sync.dma_start(ssb[:], S[:])

            def compute_gt(j):
                """gT[:, local col chunk j] -> SBUF [128, KC, CH].

                All KF vt chunks are prefetched; the psum accumulation
                runs m-outer / k-inner so each f-out chunk's psum->sbuf
                cast overlaps the next chunk's matmuls (no PE stall at
                block boundaries)."""
                gt = gtpool.tile([128, KC, CH], MMDT, tag="gt", name=f"gt{j}")
                vts = []
                for k in range(KF):
                    vt = vpool.tile([128, CH], MMDT, tag=f"vt{k}",
                                    name=f"vt{j}_{k}")
                    if j == 0:
                        # interleave the weight loads with the first
                        # column's vt loads so the first matmul isn't
                        # blocked behind the whole 3 MB of wT.
                        nc.sync.dma_start(
                            wsb[:, k, :], wT[k * 128:(k + 1) * 128, :])
                    nc.sync.dma_start(
                        vt[:], vT[k * 128:(k + 1) * 128, j * CH:(j + 1) * CH])
                    vts.append(vt)
                for m in range(KC):
                    ps = psum.tile([128, CH], F32, tag="ps", name=f"pg{j}_{m}")
                    for k in range(KF):
                        nc.tensor.matmul(
                            ps[:],
                            wsb[:, k, ts(m, 128)],
                            vts[k][:],
                            start=(k == 0), stop=(k == KF - 1))
                    nc.vector.tensor_copy(gt[:, m, :], ps[:])
                return gt

            # Own column chunk first (local col 0 == this core's rows):
            # it both feeds the A variants and serves as rhs for col 0.
            gt0 = compute_gt(0)
            A = []
            for v in range(NV):
                a = apool.tile([128, KC, CH], MMDT, tag=f"A{v}", name=f"A{v}")
                for k in range(KC):
                    nc.vector.tensor_scalar_mul(
                        a[:, k, :], gt0[:, k, :], ssb[:, v * KC + k:v * KC + k + 1])
                A.append(a)

            for j in range(NLOC):
                gt = gt0 if j == 0 else compute_gt(j)
                # full blocks use all 8 head variants; the half block
                # (j == 4) uses variants 0..3 (per-core head permutation
                # in S makes those this core's half-block heads).
                vset = list(range(N_HEADS)) if j < NLOC - 1 else \
                    list(range(NHALF))
                es = [epool.tile([128, CH], F32, tag="e", name=f"e{j}_{m}")
                      for m in range(MM)]
                for vi, v in enumerate(vset):
                    for m in range(MM):
                        ps = psum.tile([128, CH], F32, tag="ps",
                                     name=f"px{j}_{v}_{m}")
                        for k in range(KC):
                            nc.tensor.matmul(
                                ps[:],
                                A[v][:, k, ts(m, 128)],
                                gt[:, k, :],
                                start=(k == 0), stop=(k == KC - 1))
                        if vi == 0:
                            nc.scalar.activation(
                                es[m][:], ps[:],
                                mybir.ActivationFunctionType.Prelu,
                                alpha=NEG_SLOPE)
                        else:
                            t = tpool.tile([128, CH], F32, tag="tmp",
                                           name=f"t{j}_{v}_{m}")
                            nc.scalar.activation(
                                t[:], ps[:],
                                mybir.ActivationFunctionType.Prelu,
                                alpha=NEG_SLOPE)
                            nc.vector.tensor_tensor(
                                es[m][:], es[m][:], t[:], mybir.AluOpType.add)
                for m in range(MM):
                    nc.sync.dma_start(
                        out[m * 128:(m + 1) * 128, j * CH:(j + 1) * CH],
                        es[m][:])
    nc.compile()
    return nc


def _prepare_in_maps(vertex, W_vert, W_attn):
    vertex = np.ascontiguousarray(vertex, dtype=np.float32)
    W_vert = np.ascontiguousarray(W_vert, dtype=np.float32)
    W_attn = np.ascontiguousarray(W_attn, dtype=np.float32)

    vT = np.zeros((FPAD, N_NODES), dtype=NPDT_IN)
    vT[:IN_FEAT] = vertex.T.astype(NPDT_IN)
    wT = np.zeros((FPAD, HD), dtype=NPDT_IN)
    wT[:IN_FEAT] = W_vert.T.astype(NPDT_IN)

    vT_chunks = vT.reshape(FPAD, NCH, CH)
    p = np.arange(128)
    head_of = (np.arange(KC * 128).reshape(KC, 128)) // N_HIDDEN

    in_maps = []
    for i in range(NCORES):
        perm = [(i + j) % NCH for j in range(NLOC)]
        vT_core = np.ascontiguousarray(
            vT_chunks[:, perm, :].reshape(FPAD, NLOC * CH))
        # S[p, v*KC + k] = W_attn[head_v, (128k + p) // N_HIDDEN]
        # Per-core head permutation: variants 0..3 must be this core's
        # half-block heads (cores 0-3 -> heads 0-3, cores 4-7 -> 4-7).
        if i < NCORES // 2:
            heads = list(range(N_HEADS))
        else:
            heads = list(range(NHALF, N_HEADS)) + list(range(NHALF))
        S = np.empty((128, NV * KC), dtype=np.float32)
        for v, h in enumerate(heads):
            for k in range(KC):
                S[:, v * KC + k] = W_attn[h, head_of[k, p]]
        in_maps.append({"vT": vT_core, "wT": wT, "S": S})
    return in_maps


def _gather(results):
    e = np.empty((N_NODES, N_NODES), dtype=np.float32)

    def blk(i, j):
        return results[i]["out"][:, j * CH:(j + 1) * CH]

    for i in range(NCORES):
        # diagonal
        e[i * CH:(i + 1) * CH, i * CH:(i + 1) * CH] = blk(i, 0)
        # full off-diagonal blocks + mirrors
        for b in range(1, NLOC - 1):
            gj = (i + b) % NCH
            e[i * CH:(i + 1) * CH, gj * CH:(gj + 1) * CH] = blk(i, b)
            e[gj * CH:(gj + 1) * CH, i * CH:(i + 1) * CH] = blk(i, b).T
    # band-4 half blocks: e(i, i+4) = half_i + half_{i+4}.T
    for i in range(NCORES // 2):
        ii = i + NCORES // 2
        full = blk(i, NLOC - 1) + blk(ii, NLOC - 1).T
        e[i * CH:(i + 1) * CH, ii * CH:(ii + 1) * CH] = full
        e[ii * CH:(ii + 1) * CH, i * CH:(i + 1) * CH] = full.T
    return e


def _warmup():
    """The terminal occasionally reports NRT_EXEC_UNIT_UNRECOVERABLE on the
    first device touch after another process exited; a retry clears it."""
    import time
    import jax
    for _ in range(4):
        try:
            x = jax.numpy.ones((16, 16))
            np.asarray(x @ x)
            return
        except Exception:
            time.sleep(5)


def run(vertex, W_vert, W_attn, **run_kwargs):
    """Run the kernel; returns (e, BassKernelResults)."""
    if "warm" not in _CACHE:
        _warmup()
        _CACHE["warm"] = True
    if "nc" not in _CACHE:
        _CACHE["nc"] = _build()
    nc = _CACHE["nc"]
    in_maps = _prepare_in_maps(vertex, W_vert, W_attn)
    r = run_bass_kernel_spmd(nc, in_maps, core_ids=list(range(NCORES)),
                             **run_kwargs)
    return _gather(r.results), r


def kernel(vertex, W_vert, W_attn):
    e, _ = run(vertex, W_vert, W_attn)
    return e
